# revision 22
# baseline (speedup 1.0000x reference)
"""Bahdanau attention Trainium2 kernel.

Problem shapes (hardcoded): B=64, T=2048, H_enc=H_dec=512, A=512, fp32.
Strategy: data-parallel over batch across 8 cores (8 batches/core);
replicate W_enc/W_dec/V.

Per-core dataflow (per batch b):
  - enc_proj^T tile [a=128p, t] = W_encT(stationary, fp32r) x encT(moving)
    accumulated over 4 h-tiles in PSUM.
  - energy = tanh(psum + dec_proj[a,b]) fused on ACT (bias is per-partition),
    written as fp32r so the follow-up matmul accepts it.
  - scores chunk = V-dot as M=1 matmul, PSUM-accumulated over a-tiles with
    the accumulation group interleaved into the main matmul stream. All four
    t-chunks accumulate into one 4-bank PSUM tile, drained by one ACT copy
    (keeps every matmul at <=1 semaphore wait: fp32r matmuls lower to a
    fused LDW+MM with a single sync-wait slot).
  - softmax over T on one partition row (reduce_max(negate) -> Exp with
    accum_out -> reciprocal).
  - attention row normalized on DVE, broadcast to 128 partitions via a
    DRAM bounce (Tile-tracked scratch tile).
  - context = sum_t attn[t]*encT[h,t] via DVE scalar_tensor_tensor with
    accum_out (fused multiply + free-dim reduce).

Epilogue of batch b-1 is interleaved mid-way into batch b's matmul stream so
no engine stalls on the softmax dependency chain.
"""

import numpy as np

B, T, H, A = 64, 2048, 512, 512
NCORES = 8
BL = B // NCORES  # 8 batches per core
P = 128
NH = H // P  # 4 k-tiles of h
NA = A // P  # 4 tiles of a
TN = 512     # t-chunk (one PSUM bank of fp32)
NC_T = T // TN  # 4 chunks

_CACHE = {}


def _build_nc():
    import concourse.bass as bass
    import concourse.bacc as bacc
    import concourse.mybir as mybir
    from concourse.tile import TileContext

    f32 = mybir.dt.float32
    f32r = mybir.dt.float32r
    bf16 = mybir.dt.bfloat16
    AF = mybir.ActivationFunctionType
    ALU = mybir.AluOpType
    AX = mybir.AxisListType

    nc = bacc.Bacc("TRN2", target_bir_lowering=False)

    enct_d = nc.dram_tensor("enct", [BL, H, T], f32r, kind="ExternalInput")
    dect_d = nc.dram_tensor("dect", [H, BL], f32r, kind="ExternalInput")
    wenct_d = nc.dram_tensor("wenct", [H, A], f32r, kind="ExternalInput")
    wdect_d = nc.dram_tensor("wdect", [H, A], f32r, kind="ExternalInput")
    vt_d = nc.dram_tensor("vt", [A, 1], f32r, kind="ExternalInput")
    ones_d = nc.dram_tensor("ones", [1, P], f32r, kind="ExternalInput")
    ctx_d = nc.dram_tensor("ctx_out", [BL, H], f32, kind="ExternalOutput")
    attn_d = nc.dram_tensor("attn_out", [BL, T], f32, kind="ExternalOutput")

    with TileContext(nc) as tc:
        with (
            tc.tile_pool(name="consts", bufs=1) as consts,
            tc.tile_pool(name="enc", bufs=3) as encp,
            tc.tile_pool(name="energy", bufs=6) as energyp,
            tc.tile_pool(name="rows", bufs=2) as rowsp,
            tc.tile_pool(name="rows1", bufs=1) as rows1p,
            tc.tile_pool(name="reps", bufs=2) as repsp,
            tc.tile_pool(name="pe", bufs=4, space="PSUM") as pe_pool,
            tc.tile_pool(name="ps", bufs=1, space="PSUM") as ps_pool,
        ):
            # ---------------- constants (one DMA each) ----------------
            wenc_sb = consts.tile([P, NH, A], f32r, tag="wenc")
            nc.sync.dma_start(
                out=wenc_sb, in_=wenct_d.rearrange("(hi p) a -> p hi a", p=P),
            )
            vt_sb = consts.tile([P, NA, 1], f32r, tag="vt")
            nc.sync.dma_start(
                out=vt_sb, in_=vt_d.rearrange("(ai p) o -> p ai o", p=P),
            )
            wdec_sb = consts.tile([P, NH, A], f32r, tag="wdec")
            nc.sync.dma_start(
                out=wdec_sb, in_=wdect_d.rearrange("(hi p) a -> p hi a", p=P),
            )
            dect_sb = consts.tile([P, NH, BL], f32r, tag="dect")
            nc.sync.dma_start(
                out=dect_sb, in_=dect_d.rearrange("(hi p) b -> p hi b", p=P),
            )
            ones_sb = consts.tile([1, P], f32r, tag="ones")
            nc.sync.dma_start(out=ones_sb, in_=ones_d[:])

            # every matmul below may carry at most ONE semaphore wait
            # (fp32r matmuls fuse LDW+MM with a single sync-wait slot),
            # so collapse all const-DMA deps here once.
            tc.strict_bb_all_engine_barrier()

            # dec_proj[a, b] for all local batches, kept as ACT bias source
            dp_sb = []
            for ai in range(NA):
                pd = pe_pool.tile([P, TN], f32, tag="pe")
                for hi in range(NH):
                    nc.tensor.matmul(
                        pd[:, :BL],
                        lhsT=wdec_sb[:, hi, ai * P:(ai + 1) * P],
                        rhs=dect_sb[:, hi, :],
                        start=(hi == 0),
                        stop=(hi == NH - 1),
                    )
                dp = consts.tile([P, BL], f32, tag=f"dp{ai}")
                nc.vector.tensor_copy(out=dp, in_=pd[:, :BL])
                dp_sb.append(dp)

            # ------------- per-batch state carried across loop -------------
            enc_tiles = {}   # b -> [P, NH, T] tile
            score_ps = {}    # b -> [1, NC_T*TN] psum tile (4 banks)
            scores_row = {}  # b -> [1, T]
            exp_row = {}     # b -> [1, T]
            recip = {}       # b -> [1, 1]

            def ai_block(b, ai):
                """Main matmuls + tanh + V-dot accumulation for one a-tile."""
                for c in range(NC_T):
                    pe = pe_pool.tile([P, TN], f32, tag="pe")
                    for hi in range(NH):
                        nc.tensor.matmul(
                            pe,
                            lhsT=wenc_sb[:, hi, ai * P:(ai + 1) * P],
                            rhs=enc_tiles[b][:, hi, c * TN:(c + 1) * TN],
                            start=(hi == 0),
                            stop=(hi == NH - 1),
                        )
                    en = energyp.tile([P, TN], f32r, tag="en")
                    nc.scalar.activation(
                        out=en,
                        in_=pe,
                        func=AF.Tanh,
                        bias=dp_sb[ai][:, b:b + 1],
                        scale=1.0,
                    )
                    nc.tensor.matmul(
                        score_ps[b][:, c * TN:(c + 1) * TN],
                        lhsT=vt_sb[:, ai, :],
                        rhs=en,
                        start=(ai == 0),
                        stop=(ai == NA - 1),
                        skip_group_check=True,
                    )

            def scores_gather(b):
                sr = rows1p.tile([1, T], f32, tag="scores")
                scores_row[b] = sr
                # one ACT copy drains all four banks; ACT is also the engine
                # that freed them, keeping V-dot waits on a single semaphore
                nc.scalar.copy(out=sr, in_=score_ps[b])
                del score_ps[b]

            def softmax1(b):
                """negmax -> exp(+accum) -> reciprocal. Cheap row ops."""
                negmax = rowsp.tile([1, 1], f32, tag="negmax")
                nc.vector.tensor_reduce(
                    out=negmax, in_=scores_row[b], axis=AX.X, op=ALU.max,
                    negate=True,
                )
                er = rowsp.tile([1, T], f32, tag="exp")
                sumexp = rowsp.tile([1, 1], f32, tag="sumexp")
                nc.scalar.activation(
                    out=er, in_=scores_row[b], func=AF.Exp,
                    bias=negmax, scale=1.0, accum_out=sumexp,
                )
                rc = rowsp.tile([1, 1], f32, tag="recip")
                nc.vector.reciprocal(out=rc, in_=sumexp)
                exp_row[b] = er
                recip[b] = rc
                del scores_row[b]

            def epilogue2(b):
                """Replicate exp/(1/Z) across partitions on-chip, context.

                DRAM round-trips are impossible here (outputs are write-only
                at NEFF load, no internal DRAM), so replication uses a
                ones[1,128]-stationary matmul + ACT drain. Normalization is
                folded into the fused context op via the per-partition
                scalar operand.
                """
                rc = recip[b]
                # normalized attn row in one ACT op, rounded to fp32r so the
                # replicate matmuls accept it as the moving operand
                attn_row = rows1p.tile([1, T], f32r, tag="attnrow")
                nc.scalar.activation(
                    out=attn_row, in_=exp_row[b], func=AF.Identity,
                    bias=0.0, scale=rc,
                )
                nc.sync.dma_start(
                    out=attn_d[b:b + 1, :], in_=attn_row.bitcast(f32),
                )
                # replicate attn row to [128, T] via ones-stationary matmuls
                attn_rep = repsp.tile([P, T], f32, tag="attnrep")
                for c in range(NC_T):
                    pr = pe_pool.tile([P, TN], f32, tag="pe", name=f"pr_{b}_{c}")
                    nc.tensor.matmul(
                        pr,
                        lhsT=ones_sb,
                        rhs=attn_row[:, c * TN:(c + 1) * TN],
                        start=True, stop=True,
                    )
                    nc.scalar.copy(out=attn_rep[:, c * TN:(c + 1) * TN], in_=pr)
                ctx4 = rows1p.tile([P, NH], f32, tag="ctx4")
                for hi in range(NH):
                    tmp = repsp.tile([P, T], bf16, tag="tmp")
                    nc.vector.scalar_tensor_tensor(
                        out=tmp,
                        in0=enc_tiles[b][:, hi, :].bitcast(f32),
                        scalar=1.0,
                        in1=attn_rep,
                        op0=ALU.mult,
                        op1=ALU.mult,
                        accum_out=ctx4[:, hi:hi + 1],
                    )
                nc.sync.dma_start(
                    out=ctx_d[b].rearrange("(hi p) -> p hi", p=P),
                    in_=ctx4,
                )
                del enc_tiles[b], exp_row[b], recip[b]

            def prefetch(b):
                et = encp.tile([P, NH, T], f32r, tag="enc")
                nc.sync.dma_start(
                    out=et, in_=enct_d[b].rearrange("(hi p) t -> p hi t", p=P),
                )
                enc_tiles[b] = et

            # ---------------- main pipeline ----------------
            prefetch(0)
            for b in range(BL):
                if b + 1 < BL:
                    prefetch(b + 1)
                score_ps[b] = ps_pool.tile(
                    [1, NC_T * TN], f32, tag="ps", name=f"ps_{b}",
                )
                ai_block(b, 0)
                ai_block(b, 1)
                if b > 0:
                    epilogue2(b - 1)
                ai_block(b, 2)
                ai_block(b, 3)
                scores_gather(b)
                softmax1(b)
            epilogue2(BL - 1)

    nc.compile()
    return nc


def _get_nc():
    if "nc" not in _CACHE:
        _CACHE["nc"] = _build_nc()
    return _CACHE["nc"]


def make_in_maps(encoder_outputs, decoder_hidden, W_enc, W_dec, V):
    enc = np.ascontiguousarray(encoder_outputs, dtype=np.float32)
    dec = np.ascontiguousarray(decoder_hidden, dtype=np.float32)
    wenct = np.ascontiguousarray(np.asarray(W_enc, dtype=np.float32).T)
    wdect = np.ascontiguousarray(np.asarray(W_dec, dtype=np.float32).T)
    vt = np.ascontiguousarray(np.asarray(V, dtype=np.float32).reshape(A, 1))
    in_maps = []
    for c in range(NCORES):
        sl = slice(c * BL, (c + 1) * BL)
        in_maps.append({
            "enct": np.ascontiguousarray(enc[sl].transpose(0, 2, 1)),
            "dect": np.ascontiguousarray(dec[sl].T),
            "wenct": wenct,
            "wdect": wdect,
            "vt": vt,
            "ones": np.ones((1, P), dtype=np.float32),
        })
    return in_maps


def gather_results(results):
    ctx = np.concatenate([r["ctx_out"] for r in results], axis=0)
    attn = np.concatenate([r["attn_out"] for r in results], axis=0)
    return ctx, attn


def kernel(encoder_outputs, decoder_hidden, W_enc, W_dec, V):
    from concourse.bass_utils import run_bass_kernel_spmd

    nc = _get_nc()
    in_maps = make_in_maps(encoder_outputs, decoder_hidden, W_enc, W_dec, V)
    res = run_bass_kernel_spmd(nc, in_maps, list(range(NCORES)))
    return gather_results(res.results)
